# revision 20
# baseline (speedup 1.0000x reference)
"""GCNBlock (GCNConv + Dropout(eval) + ReLU) Trainium2 kernel, 8 NeuronCores.

Math: out = relu(D^-1/2 (A+I) D^-1/2 (x @ W) + b)
Factorization (aggregate-before-transform), with x pre-scaled by
ALPHA*dinv[src] on the host so every selector entry is a small exact integer
(ALPHA keeps the fp8 stream away from the subnormal floor; it is divided back
out of the dinv[dst] post-scale):
    xh[s]  = ALPHA * dinv[s] * x[s]                  (fp8 e3m4, host)
    y[d]   = dinv[d]/ALPHA * ( sum_{s in N(d) u {d}} m(s,d) * xh[s] )
    out[d] = relu( y[d] @ W + b )

Sharding: destination-node rows sharded across 8 cores (1280 rows each,
N padded 10000->10240). Per dst tile of 128 rows the host builds a PERMUTED
CONTIGUOUS stream of the source rows the tile needs (chunk 0 = the tile's own
128 rows, covering self loops and in-range edges; then the DEDUPED
out-of-range sources; zero padding) plus a matching fp8 selector table whose
entries are edge multiplicities (+I on chunk 0). The device then runs only
affine HWDGE DMAs - no dma_gather, no SWDGE descriptor generation:
    psum_y  += Sel_c.T @ stream_chunk_c        (PE, fp8e3, K=128 rows)
    y        = dinv[dst]/ALPHA * psum          (ACT, fp16)
    yT       = transpose(y)                    (PE, fp16)
    out      = relu(yT.T @ W + b)              (PE fp16 + DVE + ACT)
fp16 out rows are upcast to fp32 on the host.
"""

import os
import sys

import numpy as np

if "/opt/trn_rl_repo" not in sys.path:
    sys.path.insert(0, "/opt/trn_rl_repo")

N_NODES = 10000
DIM = 1024
N_CORES = 8
P = 128
TILES_PER_CORE = 10                      # 10240 padded rows / 8 cores / 128
N_PAD = N_CORES * TILES_PER_CORE * P     # 10240
ROWS_PER_CORE = TILES_PER_CORE * P       # 1280
ALPHA = 4.0                              # fp8 pre-scale (exactly compensated)
STREAM_FP8 = True                        # False -> fp16 stream (safe fallback)


def _stream_np_dtype():
    import ml_dtypes
    return ml_dtypes.float8_e3m4 if STREAM_FP8 else np.float16


def _host_preprocess(x, edge_index):
    """Group edges by destination tile, fold in-range sources + self loops
    into chunk 0, dedup the rest, and build the permuted row stream plus the
    fp8 selector tables."""
    sdt = _stream_np_dtype()

    src = np.asarray(edge_index[0], dtype=np.int64)
    dst = np.asarray(edge_index[1], dtype=np.int64)
    n = N_NODES
    deg = np.bincount(dst, minlength=n).astype(np.float64) + 1.0
    dinv = (1.0 / np.sqrt(deg)).astype(np.float32)

    order = np.argsort(dst, kind="stable")
    s_sorted = src[order]
    d_sorted = dst[order]

    TOT = N_PAD // P  # 80 global tiles
    bounds = np.searchsorted(d_sorted, np.arange(0, N_PAD + 1, P))
    T = TILES_PER_CORE

    # per-tile dedup pass
    tiles = []
    for t in range(TOT):
        e0, e1 = bounds[t], bounds[t + 1]
        s_t = s_sorted[e0:e1]
        d_t = (d_sorted[e0:e1] - t * P).astype(np.int64)
        inr = (s_t >= t * P) & (s_t < (t + 1) * P)
        diag = np.eye(P, dtype=np.float32)
        np.add.at(diag, (s_t[inr] - t * P, d_t[inr]), 1.0)
        uniq, inv = np.unique(s_t[~inr], return_inverse=True)
        sel = np.zeros((max(len(uniq), 1), P), np.float32)
        np.add.at(sel, (inv, d_t[~inr]), 1.0)
        tiles.append((uniq, sel, diag))

    # deal tiles to (core, slot) sorted by unique-source count so each slot's
    # static chunk count is the max over 8 similar-sized tiles (several slots
    # drop a chunk) and per-core load balances
    ks = np.array([len(u) for u, _, _ in tiles])
    order = np.argsort(-ks, kind="stable")
    assign = order.reshape(T, N_CORES).T     # [C, T] -> global tile id
    cbs = []                                 # per-slot selector block count
    for s in range(T):
        maxk = max(ks[assign[c, s]] for c in range(N_CORES))
        cbs.append(int(np.ceil(max(maxk, 1) / P)) + 1)  # + chunk 0 (own rows)
    CBMAX = max(cbs)

    # pre-scaled source rows (fp8/fp16)
    x_np = np.asarray(x, dtype=np.float32)
    xh = np.zeros((N_PAD, DIM), sdt)
    xh[:n] = (x_np * (ALPHA * dinv)[:, None]).astype(sdt)

    # permuted per-tile row stream [C, T*P, CBMAX*DIM] and selector tables
    # (both padded to CBMAX; only cbs[s] blocks are ever DMA'd)
    xp_all = np.zeros((N_CORES, T * P, CBMAX * DIM), sdt)
    sel_all = np.zeros((N_CORES, T, CBMAX, P, P), sdt)
    ddst_tbl = np.zeros((N_CORES, P, T), np.float32)
    dinv_pad = np.zeros(N_PAD, np.float32)
    dinv_pad[:n] = dinv / ALPHA
    for c in range(N_CORES):
        for s in range(T):
            t = assign[c, s]
            uniq, sel, diag = tiles[t]
            k = len(uniq)
            cb = cbs[s]
            rows = np.zeros((cb, P, DIM), sdt)
            rows[0] = xh[t * P:(t + 1) * P]
            if k > 0:
                flat = rows.reshape(cb * P, DIM)
                flat[P:P + k] = xh[uniq]
                selp = np.zeros(((cb - 1) * P, P), np.float32)
                selp[:k] = sel[:k]
                sel_all[c, s, 1:cb] = selp.reshape(cb - 1, P, P).astype(sdt)
            sel_all[c, s, 0] = diag.astype(sdt)
            xp_all[c, s * P:(s + 1) * P, :cb * DIM] = (
                rows.transpose(1, 0, 2).reshape(P, cb * DIM))
            ddst_tbl[c, :, s] = dinv_pad[t * P:(t + 1) * P]

    sel_tbl = np.ascontiguousarray(
        np.transpose(sel_all, (0, 3, 1, 2, 4)).reshape(N_CORES, P,
                                                       T * CBMAX * P)
    )  # [C, 128k, T*CBMAX*128d]

    layout = dict(CBS=cbs, CBMAX=CBMAX, assign=assign)
    return layout, xp_all, sel_tbl, ddst_tbl


def _build_bass(layout):
    import concourse.bass as bass  # noqa: F401
    import concourse.mybir as mybir
    import concourse.tile as tile
    from concourse import bacc

    dt = mybir.dt
    sdt = dt.float8e3 if STREAM_FP8 else dt.float16
    CBS = layout["CBS"]
    CB = layout["CBMAX"]
    T = TILES_PER_CORE
    KD = DIM // P  # 8 k-chunks

    nc = bacc.Bacc("TRN2", target_bir_lowering=False, debug=False,
                   num_devices=N_CORES)

    xp_d = nc.dram_tensor("xp", [T * P, CB * DIM], sdt, kind="ExternalInput").ap()
    w_d = nc.dram_tensor("w", [DIM, DIM], dt.float16, kind="ExternalInput").ap()
    b_d = nc.dram_tensor("b", [1, DIM], dt.float32, kind="ExternalInput").ap()
    sel_d = nc.dram_tensor("sel", [P, T * CB * P], sdt, kind="ExternalInput").ap()
    dd_d = nc.dram_tensor("dd", [P, T], dt.float32, kind="ExternalInput").ap()
    eye_d = nc.dram_tensor("eye", [P, P], dt.float16, kind="ExternalInput").ap()
    out_d = nc.dram_tensor("out", [ROWS_PER_CORE, DIM], dt.float16,
                           kind="ExternalOutput").ap()

    with tile.TileContext(nc) as tc:
        with (
            tc.tile_pool(name="consts", bufs=1) as consts,
            tc.tile_pool(name="g", bufs=4) as gpool,
            tc.tile_pool(name="sel", bufs=3) as selp,
            tc.tile_pool(name="y", bufs=2) as ypool,
            tc.tile_pool(name="o", bufs=2) as opool,
            tc.tile_pool(name="psy", bufs=2, space="PSUM") as ps_y,
            tc.tile_pool(name="pstr", bufs=2, space="PSUM") as ps_tr,
            tc.tile_pool(name="pso", bufs=2, space="PSUM") as ps_o,
        ):
            # resident tables (all const DMAs are issued after tile 0's
            # critical stream pieces so the first aggregation starts ASAP)
            w_sb = consts.tile([P, KD, DIM], dt.float16)
            eye_sb = consts.tile([P, P], dt.float16)
            dd_sb = consts.tile([P, T], dt.float32)
            b_sb = consts.tile([1, DIM], dt.float32)
            b_rep = consts.tile([P, DIM], dt.float32)

            # dummy matmuls on a zeroed buffer while tile 0's stream is still
            # in flight: ramps the PE clock out of its low pstate so the
            # first real aggregation runs at full speed
            warm = consts.tile([P, 512], dt.float16)
            nc.vector.memset(warm[:], 0.0)
            for _ in range(8):
                ps_w = ps_o.tile([P, 512], dt.float32, tag="po")
                nc.tensor.matmul(ps_w[:], warm[:, 0:P], warm[:],
                                 start=True, stop=True)

            def post(ti, psum_y):
                """dinv scale, transpose, transform, bias+relu, store for a
                tile whose aggregation PSUM is complete. Emitted AFTER the
                NEXT tile's aggregation matmuls so the PE never stalls on the
                ACT scale (keeps the clock ramped)."""
                # y = dinv[dst]/ALPHA * psum  (ACT copy w/ per-partition scale)
                y_sb = ypool.tile([P, DIM], dt.float16, tag="y")
                nc.scalar.mul(y_sb[:], psum_y[:], dd_sb[:, ti:ti + 1])

                # y.T chunks via PE transpose
                yT = ypool.tile([P, KD, P], dt.float16, tag="yT")
                for kc in range(KD):
                    ps_t = ps_tr.tile([P, P], dt.float16, tag="tr")
                    nc.tensor.transpose(ps_t[:], y_sb[:, kc * P:(kc + 1) * P],
                                        eye_sb[:])
                    nc.vector.tensor_copy(out=yT[:, kc, :], in_=ps_t[:])

                # out = y @ W   (two half-bank PSUM tiles so tile i+1's
                # transform never waits on tile i's bias-add)
                o_sb = opool.tile([P, DIM], dt.float16, tag="o")
                for hf in range(2):
                    ps_out = ps_o.tile([P, 512], dt.float32, tag="po")
                    for kc in range(KD):
                        nc.tensor.matmul(ps_out[:], yT[:, kc, :],
                                         w_sb[:, kc, hf * 512:(hf + 1) * 512],
                                         start=(kc == 0), stop=(kc == KD - 1))
                    # += b (fp16 out; host upcasts)
                    nc.vector.tensor_tensor(o_sb[:, hf * 512:(hf + 1) * 512],
                                            ps_out[:],
                                            b_rep[:, hf * 512:(hf + 1) * 512],
                                            mybir.AluOpType.add)
                nc.scalar.activation(o_sb[:], o_sb[:],
                                     mybir.ActivationFunctionType.Relu)
                nc.sync.dma_start(out_d[ti * P:(ti + 1) * P, :], o_sb[:])

            prev = None
            for ti in range(T):
                # contiguous permuted row stream + selector blocks; tile 0's
                # stream lands in 4 pieces so the first matmuls start early
                cb = CBS[ti]
                g_sb = gpool.tile([P, CB, DIM], sdt, tag="g")
                sel8 = selp.tile([P, CB * P], sdt, tag="sel")
                nc.sync.dma_start(sel8[:, :cb * P],
                                  sel_d[:, ti * CB * P:ti * CB * P + cb * P])
                if ti == 0:
                    cuts = [0, 2, 7, 12, cb]
                    for pc, pe in zip(cuts[:-1], cuts[1:]):
                        nc.sync.dma_start(
                            g_sb[:, pc:pe, :],
                            xp_d[ti * P:(ti + 1) * P, pc * DIM:pe * DIM])
                else:
                    nc.sync.dma_start(g_sb[:, :cb, :],
                                      xp_d[ti * P:(ti + 1) * P, :cb * DIM])
                if ti == 0:
                    nc.sync.dma_start(dd_sb[:], dd_d[:])
                    nc.sync.dma_start(eye_sb[:], eye_d[:])
                    nc.sync.dma_start(b_sb[:], b_d[:])
                    nc.gpsimd.partition_broadcast(b_rep[:], b_sb[:])
                w_view = w_d.rearrange("(ko ki) f -> ki ko f", ki=P)
                if ti == 0:
                    # W arrives in halves behind tiles 0/1's streams so tile
                    # 1's stream DMA isn't queued behind the whole 2 MB
                    nc.sync.dma_start(w_sb[:, 0:KD // 2, :],
                                      w_view[:, 0:KD // 2, :])
                elif ti == 1:
                    nc.sync.dma_start(w_sb[:, KD // 2:, :],
                                      w_view[:, KD // 2:, :])

                psum_y = ps_y.tile([P, DIM], dt.float32, tag="py")
                for c in range(cb):
                    sl = sel8[:, c * P:(c + 1) * P]
                    nc.tensor.matmul(psum_y[:, 0:512], sl, g_sb[:, c, 0:512],
                                     start=(c == 0), stop=False)
                    nc.tensor.matmul(psum_y[:, 512:1024], sl,
                                     g_sb[:, c, 512:1024],
                                     start=(c == 0), stop=(c == cb - 1))
                if prev is not None:
                    post(*prev)
                prev = (ti, psum_y)
            post(*prev)

    nc.compile()
    return nc


def _make_in_maps(x, W, b, layout, xp_all, sel_tbl, ddst_tbl):
    w_np = np.ascontiguousarray(np.asarray(W, dtype=np.float16))
    b_np = np.ascontiguousarray(np.asarray(b, dtype=np.float32)).reshape(1, DIM)
    eye = np.eye(P, dtype=np.float16)
    in_maps = []
    for c in range(N_CORES):
        in_maps.append({
            "xp": xp_all[c], "w": w_np, "b": b_np,
            "sel": sel_tbl[c], "dd": ddst_tbl[c],
            "eye": eye,
        })
    return in_maps


def _assemble(results, layout):
    assign = layout["assign"]  # [C, T] -> global tile id
    full = np.zeros((N_PAD, DIM), np.float32)
    for c in range(N_CORES):
        out_c = np.asarray(results[c]["out"])
        for s in range(TILES_PER_CORE):
            t = assign[c, s]
            full[t * P:(t + 1) * P] = out_c[s * P:(s + 1) * P]
    return np.ascontiguousarray(full[:N_NODES])


def kernel(x, edge_index, W, b):
    from concourse import bass_utils

    layout, xp_all, sel_tbl, ddst_tbl = _host_preprocess(x, edge_index)
    nc = _build_bass(layout)
    in_maps = _make_in_maps(x, W, b, layout, xp_all, sel_tbl, ddst_tbl)
    res = bass_utils.run_bass_kernel_spmd(nc, in_maps, core_ids=list(range(N_CORES)))
    return _assemble(res.results, layout)


# revision 21
# speedup vs baseline: 1.1713x; 1.1713x over previous
"""GCNBlock (GCNConv + Dropout(eval) + ReLU) Trainium2 kernel, 8 NeuronCores.

Math: out = relu(D^-1/2 (A+I) D^-1/2 (x @ W) + b)
Factorization (aggregate-before-transform), with x pre-scaled by
ALPHA*dinv[src] on the host so every selector entry is a small exact integer
(ALPHA keeps the fp8 stream away from the subnormal floor; it is divided back
out of the dinv[dst] post-scale):
    xh[s]  = ALPHA * dinv[s] * x[s]                  (fp8 e3m4, host)
    y[d]   = dinv[d]/ALPHA * ( sum_{s in N(d) u {d}} m(s,d) * xh[s] )
    out[d] = relu( y[d] @ W + b )

Sharding: destination-node rows sharded across 8 cores (1280 rows each,
N padded 10000->10240). Per dst tile of 128 rows the host builds a PERMUTED
CONTIGUOUS stream of the source rows the tile needs (chunk 0 = the tile's own
128 rows, covering self loops and in-range edges; then the DEDUPED
out-of-range sources; zero padding) plus a matching fp8 selector table whose
entries are edge multiplicities (+I on chunk 0). The device then runs only
affine HWDGE DMAs - no dma_gather, no SWDGE descriptor generation:
    psum_y  += Sel_c.T @ stream_chunk_c        (PE, fp8e3, K=128 rows)
    y        = dinv[dst]/ALPHA * psum          (ACT, fp16)
    yT       = transpose(y)                    (PE, fp16)
    out      = relu(yT.T @ W + b)              (PE fp16 + DVE + ACT)
fp16 out rows are upcast to fp32 on the host.
"""

import os
import sys

import numpy as np

if "/opt/trn_rl_repo" not in sys.path:
    sys.path.insert(0, "/opt/trn_rl_repo")

N_NODES = 10000
DIM = 1024
N_CORES = 8
P = 128
TILES_PER_CORE = 10                      # 10240 padded rows / 8 cores / 128
N_PAD = N_CORES * TILES_PER_CORE * P     # 10240
ROWS_PER_CORE = TILES_PER_CORE * P       # 1280
ALPHA = 4.0                              # fp8 pre-scale (exactly compensated)
STREAM_FP8 = True                        # False -> fp16 stream (safe fallback)


def _stream_np_dtype():
    import ml_dtypes
    return ml_dtypes.float8_e3m4 if STREAM_FP8 else np.float16


def _host_preprocess(x, edge_index):
    """Group edges by destination tile, fold in-range sources + self loops
    into chunk 0, dedup the rest, and build the permuted row stream plus the
    fp8 selector tables."""
    sdt = _stream_np_dtype()

    src = np.asarray(edge_index[0], dtype=np.int64)
    dst = np.asarray(edge_index[1], dtype=np.int64)
    n = N_NODES
    deg = np.bincount(dst, minlength=n).astype(np.float64) + 1.0
    dinv = (1.0 / np.sqrt(deg)).astype(np.float32)

    order = np.argsort(dst, kind="stable")
    s_sorted = src[order]
    d_sorted = dst[order]

    TOT = N_PAD // P  # 80 global tiles
    bounds = np.searchsorted(d_sorted, np.arange(0, N_PAD + 1, P))
    T = TILES_PER_CORE

    # per-tile dedup pass
    tiles = []
    for t in range(TOT):
        e0, e1 = bounds[t], bounds[t + 1]
        s_t = s_sorted[e0:e1]
        d_t = (d_sorted[e0:e1] - t * P).astype(np.int64)
        inr = (s_t >= t * P) & (s_t < (t + 1) * P)
        diag = np.eye(P, dtype=np.float32)
        np.add.at(diag, (s_t[inr] - t * P, d_t[inr]), 1.0)
        uniq, inv = np.unique(s_t[~inr], return_inverse=True)
        sel = np.zeros((max(len(uniq), 1), P), np.float32)
        np.add.at(sel, (inv, d_t[~inr]), 1.0)
        tiles.append((uniq, sel, diag))

    # deal tiles to (core, slot) sorted by unique-source count so each slot's
    # static chunk count is the max over 8 similar-sized tiles (several slots
    # drop a chunk) and per-core load balances
    ks = np.array([len(u) for u, _, _ in tiles])
    order = np.argsort(-ks, kind="stable")
    assign = order.reshape(T, N_CORES).T     # [C, T] -> global tile id
    cbs = []                                 # per-slot selector block count
    for s in range(T):
        maxk = max(ks[assign[c, s]] for c in range(N_CORES))
        cbs.append(int(np.ceil(max(maxk, 1) / P)) + 1)  # + chunk 0 (own rows)
    CBMAX = max(cbs)

    # pre-scaled source rows (fp8/fp16)
    x_np = np.asarray(x, dtype=np.float32)
    xh = np.zeros((N_PAD, DIM), sdt)
    xh[:n] = (x_np * (ALPHA * dinv)[:, None]).astype(sdt)

    # permuted per-tile row stream [C, T*P, CBMAX*DIM] and selector tables
    # (both padded to CBMAX; only cbs[s] blocks are ever DMA'd)
    xp_all = np.zeros((N_CORES, T * P, CBMAX * DIM), sdt)
    sel_all = np.zeros((N_CORES, T, CBMAX, P, P), sdt)
    ddst_tbl = np.zeros((N_CORES, P, T), np.float32)
    dinv_pad = np.zeros(N_PAD, np.float32)
    dinv_pad[:n] = dinv / ALPHA
    for c in range(N_CORES):
        for s in range(T):
            t = assign[c, s]
            uniq, sel, diag = tiles[t]
            k = len(uniq)
            cb = cbs[s]
            rows = np.zeros((cb, P, DIM), sdt)
            rows[0] = xh[t * P:(t + 1) * P]
            if k > 0:
                flat = rows.reshape(cb * P, DIM)
                flat[P:P + k] = xh[uniq]
                selp = np.zeros(((cb - 1) * P, P), np.float32)
                selp[:k] = sel[:k]
                sel_all[c, s, 1:cb] = selp.reshape(cb - 1, P, P).astype(sdt)
            sel_all[c, s, 0] = diag.astype(sdt)
            xp_all[c, s * P:(s + 1) * P, :cb * DIM] = (
                rows.transpose(1, 0, 2).reshape(P, cb * DIM))
            ddst_tbl[c, :, s] = dinv_pad[t * P:(t + 1) * P]

    sel_tbl = np.ascontiguousarray(
        np.transpose(sel_all, (0, 3, 1, 2, 4)).reshape(N_CORES, P,
                                                       T * CBMAX * P)
    )  # [C, 128k, T*CBMAX*128d]

    layout = dict(CBS=cbs, CBMAX=CBMAX, assign=assign)
    return layout, xp_all, sel_tbl, ddst_tbl


def _build_bass(layout):
    import concourse.bass as bass  # noqa: F401
    import concourse.mybir as mybir
    import concourse.tile as tile
    from concourse import bacc

    dt = mybir.dt
    sdt = dt.float8e3 if STREAM_FP8 else dt.float16
    CBS = layout["CBS"]
    CB = layout["CBMAX"]
    T = TILES_PER_CORE
    KD = DIM // P  # 8 k-chunks

    nc = bacc.Bacc("TRN2", target_bir_lowering=False, debug=False,
                   num_devices=N_CORES)

    xp_d = nc.dram_tensor("xp", [T * P, CB * DIM], sdt, kind="ExternalInput").ap()
    w_d = nc.dram_tensor("w", [DIM, DIM], dt.float16, kind="ExternalInput").ap()
    b_d = nc.dram_tensor("b", [1, DIM], dt.float32, kind="ExternalInput").ap()
    sel_d = nc.dram_tensor("sel", [P, T * CB * P], sdt, kind="ExternalInput").ap()
    dd_d = nc.dram_tensor("dd", [P, T], dt.float32, kind="ExternalInput").ap()
    eye_d = nc.dram_tensor("eye", [P, P], dt.float16, kind="ExternalInput").ap()
    out_d = nc.dram_tensor("out", [ROWS_PER_CORE, DIM], dt.float16,
                           kind="ExternalOutput").ap()

    with tile.TileContext(nc) as tc:
        with (
            tc.tile_pool(name="consts", bufs=1) as consts,
            tc.tile_pool(name="g", bufs=4) as gpool,
            tc.tile_pool(name="sel", bufs=3) as selp,
            tc.tile_pool(name="y", bufs=2) as ypool,
            tc.tile_pool(name="o", bufs=2) as opool,
            tc.tile_pool(name="psy", bufs=2, space="PSUM") as ps_y,
            tc.tile_pool(name="pstr", bufs=2, space="PSUM") as ps_tr,
            tc.tile_pool(name="pso", bufs=2, space="PSUM") as ps_o,
        ):
            # resident tables (all const DMAs are issued after tile 0's
            # critical stream pieces so the first aggregation starts ASAP)
            w_sb = consts.tile([P, KD, DIM], dt.float16)
            eye_sb = consts.tile([P, P], dt.float16)
            dd_sb = consts.tile([P, T], dt.float32)
            b_sb = consts.tile([1, DIM], dt.float32)
            b_rep = consts.tile([P, DIM], dt.float32)

            def post(ti, psum_y):
                """dinv scale, transpose, transform, bias+relu, store for a
                tile whose aggregation PSUM is complete. Emitted AFTER the
                NEXT tile's aggregation matmuls so the PE never stalls on the
                ACT scale (keeps the clock ramped)."""
                # y = dinv[dst]/ALPHA * psum  (ACT copy w/ per-partition scale)
                y_sb = ypool.tile([P, DIM], dt.float16, tag="y")
                nc.scalar.mul(y_sb[:], psum_y[:], dd_sb[:, ti:ti + 1])

                # y.T chunks via PE transpose
                yT = ypool.tile([P, KD, P], dt.float16, tag="yT")
                for kc in range(KD):
                    ps_t = ps_tr.tile([P, P], dt.float16, tag="tr")
                    nc.tensor.transpose(ps_t[:], y_sb[:, kc * P:(kc + 1) * P],
                                        eye_sb[:])
                    nc.vector.tensor_copy(out=yT[:, kc, :], in_=ps_t[:])

                # out = y @ W   (two half-bank PSUM tiles so tile i+1's
                # transform never waits on tile i's bias-add)
                o_sb = opool.tile([P, DIM], dt.float16, tag="o")
                for hf in range(2):
                    ps_out = ps_o.tile([P, 512], dt.float32, tag="po")
                    for kc in range(KD):
                        nc.tensor.matmul(ps_out[:], yT[:, kc, :],
                                         w_sb[:, kc, hf * 512:(hf + 1) * 512],
                                         start=(kc == 0), stop=(kc == KD - 1))
                    # += b (fp16 out; host upcasts)
                    nc.vector.tensor_tensor(o_sb[:, hf * 512:(hf + 1) * 512],
                                            ps_out[:],
                                            b_rep[:, hf * 512:(hf + 1) * 512],
                                            mybir.AluOpType.add)
                nc.scalar.activation(o_sb[:], o_sb[:],
                                     mybir.ActivationFunctionType.Relu)
                nc.sync.dma_start(out_d[ti * P:(ti + 1) * P, :], o_sb[:])

            prev = None
            for ti in range(T):
                # contiguous permuted row stream + selector blocks; tile 0's
                # stream lands in 4 pieces so the first matmuls start early
                cb = CBS[ti]
                g_sb = gpool.tile([P, CB, DIM], sdt, tag="g")
                sel8 = selp.tile([P, CB * P], sdt, tag="sel")
                nc.sync.dma_start(sel8[:, :cb * P],
                                  sel_d[:, ti * CB * P:ti * CB * P + cb * P])
                if ti == 0:
                    cuts = [0, 2, 7, 12, cb]
                    for pc, pe in zip(cuts[:-1], cuts[1:]):
                        nc.sync.dma_start(
                            g_sb[:, pc:pe, :],
                            xp_d[ti * P:(ti + 1) * P, pc * DIM:pe * DIM])
                else:
                    nc.sync.dma_start(g_sb[:, :cb, :],
                                      xp_d[ti * P:(ti + 1) * P, :cb * DIM])
                if ti == 0:
                    nc.sync.dma_start(dd_sb[:], dd_d[:])
                    nc.sync.dma_start(eye_sb[:], eye_d[:])
                    nc.sync.dma_start(b_sb[:], b_d[:])
                    nc.gpsimd.partition_broadcast(b_rep[:], b_sb[:])
                w_view = w_d.rearrange("(ko ki) f -> ki ko f", ki=P)
                if ti == 0:
                    # W arrives in halves behind tiles 0/1's streams so tile
                    # 1's stream DMA isn't queued behind the whole 2 MB
                    nc.sync.dma_start(w_sb[:, 0:KD // 2, :],
                                      w_view[:, 0:KD // 2, :])
                elif ti == 1:
                    nc.sync.dma_start(w_sb[:, KD // 2:, :],
                                      w_view[:, KD // 2:, :])

                psum_y = ps_y.tile([P, DIM], dt.float32, tag="py")
                for c in range(cb):
                    sl = sel8[:, c * P:(c + 1) * P]
                    nc.tensor.matmul(psum_y[:, 0:512], sl, g_sb[:, c, 0:512],
                                     start=(c == 0), stop=False)
                    nc.tensor.matmul(psum_y[:, 512:1024], sl,
                                     g_sb[:, c, 512:1024],
                                     start=(c == 0), stop=(c == cb - 1))
                if prev is not None:
                    post(*prev)
                prev = (ti, psum_y)
            post(*prev)

    nc.compile()
    return nc


def _make_in_maps(x, W, b, layout, xp_all, sel_tbl, ddst_tbl):
    w_np = np.ascontiguousarray(np.asarray(W, dtype=np.float16))
    b_np = np.ascontiguousarray(np.asarray(b, dtype=np.float32)).reshape(1, DIM)
    eye = np.eye(P, dtype=np.float16)
    in_maps = []
    for c in range(N_CORES):
        in_maps.append({
            "xp": xp_all[c], "w": w_np, "b": b_np,
            "sel": sel_tbl[c], "dd": ddst_tbl[c],
            "eye": eye,
        })
    return in_maps


def _assemble(results, layout):
    assign = layout["assign"]  # [C, T] -> global tile id
    full = np.zeros((N_PAD, DIM), np.float32)
    for c in range(N_CORES):
        out_c = np.asarray(results[c]["out"])
        for s in range(TILES_PER_CORE):
            t = assign[c, s]
            full[t * P:(t + 1) * P] = out_c[s * P:(s + 1) * P]
    return np.ascontiguousarray(full[:N_NODES])


def kernel(x, edge_index, W, b):
    from concourse import bass_utils

    layout, xp_all, sel_tbl, ddst_tbl = _host_preprocess(x, edge_index)
    nc = _build_bass(layout)
    in_maps = _make_in_maps(x, W, b, layout, xp_all, sel_tbl, ddst_tbl)
    res = bass_utils.run_bass_kernel_spmd(nc, in_maps, core_ids=list(range(N_CORES)))
    return _assemble(res.results, layout)
